# revision 26
# baseline (speedup 1.0000x reference)
"""Trainium2 Bass kernel for CrossTransformerBlock (KNN sparse cross-attention).

Problem shapes (hardcoded): b=4, nQ=4096, N=2048, dim_inp=256, dim=128, K=16.
Sharding: 8 cores = (batch b) x (query halves of 2048). Each core handles one
batch's full point set and a 2048-query slice; output is gathered on host.

Per-core algorithm (feature-major [feat_partition, token_free] on chip, token
order j = q*16 + s), software-pipelined across 16 query row-tiles so the
dist->topk->idx->gather->MLP phases of different tiles overlap on their
respective engines (PE / ACT / DVE / Pool / DMA):

  score[q,n] = 2*q.x - |x|^2  (row-monotone == -dist), fp32r single-pass
  matmul -> fp16 scores -> top-16 per query via DVE max8/max_index/
  match_replace (fp16, two rounds).
  Dense per-point table precomputed once per core, staged in DRAM as bf16
  rows [v | w | c] (384 cols):
     v = points @ w_vs                  (value vectors)
     w = -(points @ w_ks) @ gw1         (attn-MLP L1 key term)
     c = -xyz @ dw1                     (pos-MLP L1 neighbor term)
  One dma_gather(transpose=True) per row-tile pulls all 2048 neighbor rows
  feature-major: G[p, 0, j]=v, G[p, 1, j]=w, G[p, 2, j]=c. MLP chain per
  512-token chunk (gw1 folded through dw2: W2 = dw2 @ gw1):
     h1  = relu(a + c + db1)        a = xyz_q@dw1 (slot-bcast rhs)
     y'  = dw2.T @ h1 + v           (stays in PSUM; db2 deferred)
     hat = relu(W2.T @ h1 + w + (u + gb1 + gw1.T@db2))
     e   = exp(gw2.T @ hat + gb2)
     res = (sum_s e*y' + db2*sum_s e + e_g*v_g) / (sum_s e + e_g)
  Global slot via per-batch constant vectors (e_g, v_g). Output is stored
  feature-major [dim, q]; the host transposes.
"""

import os
import sys

import numpy as np

for _p in ("/opt/trn_rl_repo", "/root/.axon_site/_ro/trn_rl_repo"):
    if os.path.isdir(_p) and _p not in sys.path:
        sys.path.insert(0, _p)

import concourse.bass as bass  # noqa: E402
import concourse.tile as tile  # noqa: E402
from concourse import bacc, mybir  # noqa: E402
from concourse.masks import make_identity  # noqa: E402

F32 = mybir.dt.float32
F32R = mybir.dt.float32r
F16 = mybir.dt.float16
BF16 = mybir.dt.bfloat16
U16 = mybir.dt.uint16
I16 = mybir.dt.int16
AF = mybir.ActivationFunctionType
ALU = mybir.AluOpType
AX = mybir.AxisListType

B, NQ, NPT, DI, D = 4, 4096, 2048, 256, 128
KNN = 16
NCORES = 8
QPC = B * NQ // NCORES  # 2048 queries per core
NT = QPC // 128  # 16 row-tiles
TBL_W = 3 * D  # 384 bf16: [v | w | c]
TOK = 128 * KNN  # 2048 tokens per row-tile (q-major: j = q*16 + s)
NEG_BIG_F16 = -60000.0
LA = 3  # pipeline lookahead (tiles)
CH = 512  # MLP chunk = 512 tokens = 32 queries
NCH = TOK // CH  # 4 chunks per tile


def _emit(tc: tile.TileContext, io: dict):
    nc = tc.nc
    xyz_q, xyz, points, lat = io["xyz_q"], io["xyz"], io["points"], io["lat"]
    out = io["out"]
    tbl = io["tbl"]
    ptbf = io["ptbf"]

    from contextlib import ExitStack

    with ExitStack() as ctx:
        const = ctx.enter_context(tc.tile_pool(name="const", bufs=1))
        setup = ctx.enter_context(tc.tile_pool(name="setup", bufs=3))
        loop = ctx.enter_context(tc.tile_pool(name="loop", bufs=2))
        # PSUM budget (8 banks of [128,512]f32): ps_c1 2 + ps_me 2 + ps_p 2
        # + ps_d ([128,1024]x1) 2 = 8.
        psum = ctx.enter_context(tc.tile_pool(name="psum", bufs=2, space="PSUM"))

        # ---------------- constants / weights ----------------
        ident_b = const.tile([128, 128], BF16)
        make_identity(nc, ident_b[:])
        ident_f = const.tile([128, 128], F32)
        make_identity(nc, ident_f[:])
        # REP[k, m] = 1 if m%16==k: replicates a [16, x] rhs into [128, x]
        rep16h = const.tile([16, 128], F16)
        for g in range(8):
            nc.vector.tensor_copy(rep16h[:, 16 * g : 16 * (g + 1)], ident_f[0:16, 0:16])

        def load_f32(name, shape):
            t = const.tile(shape, F32, name=name + "_f", tag=name)
            nc.sync.dma_start(out=t[:], in_=io[name][:, :])
            return t

        def load_bf16(name, shape, tag=None):
            t = const.tile(shape, BF16, name=name + "_b", tag=tag or (name + "_b"))
            nc.gpsimd.dma_start(out=t[:], in_=io[name][:, :])
            return t

        gw1f = load_f32("gw1", [128, 128])
        gw2f = load_f32("gw2", [128, 128])
        dw2f = load_f32("dw2", [128, 128])
        gw2b = load_bf16("gw2", [128, 128])
        dw2b = load_bf16("dw2", [128, 128])
        dw1b = const.tile([3, 128], BF16)
        nc.gpsimd.dma_start(out=dw1b[:], in_=io["dw1"][:, :])
        ndw1b = const.tile([3, 128], BF16)
        nc.vector.tensor_scalar_mul(ndw1b[:], dw1b[:], -1.0)

        # biases as [128,1] partition vectors
        def load_bias_col(name):
            t = const.tile([128, 1], F32, name=name + "_c", tag=name + "_c")
            nc.sync.dma_start(out=t[:], in_=io[name][:, :].rearrange("a b -> b a"))
            return t

        db1c = load_bias_col("db1")
        db2c = load_bias_col("db2")
        gb1c = load_bias_col("gb1")
        gb2c = load_bias_col("gb2")

        wq = [const.tile([128, 128], F32, name=f"wq{h}", tag=f"wqs{h}") for h in range(2)]
        wk = [const.tile([128, 128], F32, name=f"wk{h}", tag=f"wks{h}") for h in range(2)]
        wkg = [const.tile([128, 128], F32, name=f"wkg{h}", tag=f"wkg{h}") for h in range(2)]
        wvg = [const.tile([128, 128], F32, name=f"wvg{h}", tag=f"wvg{h}") for h in range(2)]
        wvw = [const.tile([128, 256], BF16, name=f"wvw{h}", tag=f"wvw{h}") for h in range(2)]
        for h in range(2):
            sl = slice(h * 128, (h + 1) * 128)
            nc.sync.dma_start(out=wq[h][:], in_=io["w_qs"][sl, :])
            nc.sync.dma_start(out=wk[h][:], in_=io["w_ks"][sl, :])
            nc.sync.dma_start(out=wkg[h][:], in_=io["w_kg"][sl, :])
            nc.sync.dma_start(out=wvg[h][:], in_=io["w_vg"][sl, :])
            nc.gpsimd.dma_start(out=wvw[h][:, 0:128], in_=io["w_vs"][sl, :])

        lat_c = [const.tile([128, 1], F32, name=f"latc{h}", tag=f"lat{h}") for h in range(2)]
        for h in range(2):
            nc.sync.dma_start(
                out=lat_c[h][:],
                in_=lat[0:1, h * 128 : (h + 1) * 128].rearrange("a b -> b a"),
            )

        # ---------------- W_w = -(w_ks @ gw1), W2 = dw2 @ gw1 (bf16) --------
        wksT = [setup.tile([128, 128], F32, name=f"wksT{h}", tag=f"wksT{h}") for h in range(2)]
        for h in range(2):
            ps = psum.tile([128, 512], F32, tag="ps_c1")
            nc.tensor.matmul(
                out=ps[:, 0:128], lhsT=wk[h][:], rhs=ident_f[:],
                is_transpose=True, start=True, stop=True,
            )
            nc.scalar.activation(out=wksT[h][:], in_=ps[:, 0:128], func=AF.Copy)
        for h in range(2):
            ps = psum.tile([128, 512], F32, tag="ps_c1")
            nc.tensor.matmul(
                out=ps[:, 0:128], lhsT=wksT[h][:], rhs=gw1f[:],
                start=True, stop=True,
            )
            nc.scalar.activation(out=wvw[h][:, 128:256], in_=ps[:, 0:128], func=AF.Copy, scale=-1.0)

        # W2 = dw2 @ gw1  (lhsT = dw2.T via PE transpose)
        dw2T = setup.tile([128, 128], F32, tag="dw2T")
        ps = psum.tile([128, 512], F32, tag="ps_c1")
        nc.tensor.matmul(
            out=ps[:, 0:128], lhsT=dw2f[:], rhs=ident_f[:],
            is_transpose=True, start=True, stop=True,
        )
        nc.scalar.activation(out=dw2T[:], in_=ps[:, 0:128], func=AF.Copy)
        W2b = const.tile([128, 128], BF16)
        ps = psum.tile([128, 512], F32, tag="ps_c1")
        nc.tensor.matmul(
            out=ps[:, 0:128], lhsT=dw2T[:], rhs=gw1f[:], start=True, stop=True,
        )
        nc.scalar.activation(out=W2b[:], in_=ps[:, 0:128], func=AF.Copy)

        # ---------------- per-tile setup + per-point table -> DRAM ----------
        rhs_aug = const.tile([4, 2048], F32)  # [2x;2y;2z;-|x|^2]
        xyzTb = const.tile([3, 2048], BF16)  # raw xyz^T (bf16, c table)
        xyzqTf = const.tile([4, 2048], F32)  # [qx;qy;qz;1]
        xyzqTb = const.tile([3, 2048], BF16)  # raw xyz_q^T (bf16, a-term)
        sqcols = const.tile([128, NT], F32)
        q2cols = const.tile([128, NT], F32)  # |q|^2 per query (score shift)
        # row 3 must be 1.0; fill the whole tile, transpose drains overwrite.
        nc.vector.memset(xyzqTf[:], 1.0)
        ptT = [const.tile([128, 2048], BF16, name=f"ptT{h}", tag=f"ptT{h}") for h in range(2)]

        def setup_a(nt):
            sl = slice(nt * 128, (nt + 1) * 128)
            x_t = setup.tile([128, 3], F32, tag="x_t")
            q_t = setup.tile([128, 3], F32, tag="q_t")
            p_t = setup.tile([128, 256], F32, tag="p_t")
            nc.sync.dma_start(out=x_t[:], in_=xyz[sl, :])
            nc.sync.dma_start(out=q_t[:], in_=xyz_q[sl, :])
            nc.sync.dma_start(out=p_t[:], in_=points[sl, :])
            pb = setup.tile([128, 256], BF16, tag="pb")
            nc.vector.tensor_copy(pb[:], p_t[:])
            nc.sync.dma_start(out=ptbf[sl, :], in_=pb[:])
            ps = psum.tile([128, 512], F32, tag="ps_c1")
            nc.tensor.matmul(
                out=ps[0:3, 256:384], lhsT=x_t[:], rhs=ident_f[:],
                is_transpose=True, start=True, stop=True,
            )
            nc.tensor.matmul(
                out=ps[0:3, 384:512], lhsT=q_t[:], rhs=ident_f[:],
                is_transpose=True, start=True, stop=True,
            )
            nc.scalar.activation(
                out=rhs_aug[0:3, sl], in_=ps[0:3, 256:384], func=AF.Copy, scale=2.0
            )
            nc.scalar.activation(out=xyzTb[:, sl], in_=ps[0:3, 256:384], func=AF.Copy)
            nc.scalar.activation(out=xyzqTf[0:3, sl], in_=ps[0:3, 384:512], func=AF.Copy)
            nc.scalar.activation(out=xyzqTb[:, sl], in_=ps[0:3, 384:512], func=AF.Copy)
            s3 = setup.tile([128, 3], F32, tag="s3")
            nc.vector.tensor_mul(s3[:], x_t[:], x_t[:])
            nc.vector.tensor_reduce(
                out=sqcols[:, nt : nt + 1], in_=s3[:], axis=AX.X, op=ALU.add
            )
            s3q = setup.tile([128, 3], F32, tag="s3q")
            nc.vector.tensor_mul(s3q[:], q_t[:], q_t[:])
            nc.vector.tensor_reduce(
                out=q2cols[:, nt : nt + 1], in_=s3q[:], axis=AX.X, op=ALU.add
            )

        def setup_b(nt):
            sl = slice(nt * 128, (nt + 1) * 128)
            # table rows: [v | w] fused 256-col rhs + [c]
            ps2 = psum.tile([128, 512], F32, tag="ps_p")
            for h in range(2):
                nc.tensor.matmul(
                    out=ps2[:, 0:256], lhsT=ptT[h][:, sl], rhs=wvw[h][:],
                    start=(h == 0), stop=(h == 1),
                )
            nc.tensor.matmul(
                out=ps2[:, 256:384], lhsT=xyzTb[:, sl], rhs=ndw1b[:],
                start=True, stop=True,
            )
            tb = setup.tile([128, TBL_W], BF16, tag="tb")
            nc.scalar.activation(out=tb[:], in_=ps2[:, 0:TBL_W], func=AF.Copy)
            nc.sync.dma_start(out=tbl[sl, :], in_=tb[:])

        for nt in range(NT):
            setup_a(nt)

        # -|x|^2 -> rhs_aug row 3  ([128,16] -> [16,128] -> [1,2048])
        ps = psum.tile([128, 256], F32, tag="ps_me")
        nc.tensor.matmul(
            out=ps[0:NT, 0:128], lhsT=sqcols[:], rhs=ident_f[:],
            is_transpose=True, start=True, stop=True,
        )
        negsq = setup.tile([NT, 128], F32, tag="negsq")
        nc.scalar.activation(out=negsq[:], in_=ps[0:NT, 0:128], func=AF.Copy, scale=-1.0)
        nc.gpsimd.dma_start(out=rhs_aug[3:4, :], in_=negsq[:])
        negq2 = const.tile([128, NT], F32)
        nc.vector.tensor_scalar_mul(negq2[:], q2cols[:], -1.0)

        # ---------------- per-batch global-slot vectors ----------------
        def matvec_col(chunks_lhsT, rhs_cols, tag, scale=1.0, func=AF.Copy, bias=0.0):
            ps = psum.tile([128, 256], F32, tag="ps_me")
            n = len(chunks_lhsT)
            for h in range(n):
                nc.tensor.matmul(
                    out=ps[:, 0:1], lhsT=chunks_lhsT[h][:], rhs=rhs_cols[h][:],
                    start=(h == 0), stop=(h == n - 1),
                )
            t = const.tile([128, 1], F32, name=tag, tag=tag)
            nc.scalar.activation(out=t[:], in_=ps[:, 0:1], func=func, bias=bias, scale=scale)
            return t

        qac = matvec_col(wq, lat_c, "qac")
        kgc = matvec_col(wkg, lat_c, "kgc")
        vgc = matvec_col(wvg, lat_c, "vgc")
        u_c = matvec_col([gw1f], [qac], "u_c")
        wg_c = matvec_col([gw1f], [kgc], "wg_c", scale=-1.0)
        c2_c = matvec_col([gw1f], [db2c], "c2_c")  # gw1.T @ db2
        gb1u = const.tile([128, 1], F32)
        nc.vector.tensor_add(gb1u[:], gb1c[:], u_c[:])
        gb1uw = const.tile([128, 1], F32)  # gb1 + u + gw1.T@db2 (hat bias)
        nc.vector.tensor_add(gb1uw[:], gb1u[:], c2_c[:])
        uw = const.tile([128, 1], F32)
        nc.vector.tensor_add(uw[:], u_c[:], wg_c[:])
        hg_c = const.tile([128, 1], F32)
        nc.scalar.activation(out=hg_c[:], in_=uw[:], func=AF.Relu, bias=gb1c[:])
        eg_c = matvec_col([gw2f], [hg_c], "eg_c", func=AF.Exp, bias=gb2c[:])
        egvg = const.tile([128, 1], F32)
        nc.vector.tensor_mul(egvg[:], eg_c[:], vgc[:])

        # ---------------- pipelined main loop over query row-tiles ----------

        def phase_dist_topk(t):
            """dist row-group matmuls (fp32) -> shifted fp32 scores -> top-16."""
            dist = loop.tile([128, 2048], F32, tag="dist", bufs=3)
            for hf in range(2):
                ps_d = psum.tile([128, 1024], F32, tag="ps_d", bufs=1)
                for c in range(2):
                    nc.tensor.matmul(
                        out=ps_d[:, c * 512 : (c + 1) * 512],
                        lhsT=xyzqTf[:, t * 128 : (t + 1) * 128],
                        rhs=rhs_aug[:, hf * 1024 + c * 512 : hf * 1024 + (c + 1) * 512],
                        start=True, stop=True,
                    )
                nc.scalar.activation(
                    out=dist[:, hf * 1024 : (hf + 1) * 1024], in_=ps_d[:],
                    func=AF.Identity, bias=negq2[:, t : t + 1],
                )

            mx = loop.tile([128, 16], F32, tag="mx", bufs=4)
            idx = loop.tile([128, 16], U16, tag="idx", bufs=4)
            nc.vector.max(out=mx[:, 0:8], in_=dist[:])
            nc.vector.max_index(out=idx[:, 0:8], in_max=mx[:, 0:8], in_values=dist[:])
            nc.vector.match_replace(
                out=dist[:], in_to_replace=mx[:, 0:8], in_values=dist[:],
                imm_value=-3.0e38,
            )
            nc.vector.max(out=mx[:, 8:16], in_=dist[:])
            nc.vector.max_index(out=idx[:, 8:16], in_max=mx[:, 8:16], in_values=dist[:])
            idxf = loop.tile([128, 16], F32, tag="idxf", bufs=4)
            nc.vector.tensor_copy(idxf[:], idx[:])
            return idxf

        def phase_idx(t, idxf):
            """wrap indices for dma_gather: [128,16] -> [16,128] -> replicate."""
            ps_x = psum.tile([128, 256], F32, tag="ps_me")
            nc.tensor.matmul(
                out=ps_x[0:16, 0:128], lhsT=idxf[:], rhs=ident_f[:],
                is_transpose=True, start=True, stop=True,
            )
            idxT = loop.tile([16, 128], F16, tag="idxT", bufs=3)
            nc.vector.tensor_copy(idxT[:], ps_x[0:16, 0:128])
            nc.tensor.matmul(
                out=ps_x[:, 128:256], lhsT=rep16h[:],
                rhs=idxT[:], start=True, stop=True,
            )
            wrapped = loop.tile([128, 128], I16, tag="wrapped", bufs=4)
            nc.vector.tensor_copy(wrapped[:], ps_x[:, 128:256])
            return wrapped

        def phase_gather(t, wrapped):
            G = loop.tile([128, 3, TOK], BF16, tag="G", bufs=LA + 1)
            nc.gpsimd.dma_gather(
                out_ap=G[:], in_ap=tbl[:, :], idxs_ap=wrapped[:],
                num_idxs=TOK, num_idxs_reg=TOK, elem_size=TBL_W, transpose=True,
                single_packet=False,
            )
            return G

        def phase_mlp(t, G, ks, resf=None):
            """MLP + per-channel softmax for chunks `ks`; returns resf.

            Within each chunk the three identity adds run back-to-back so the
            identity weight is loaded once (a, ic, iv, iw, dw2, W2, gw2)."""
            if resf is None:
                resf = (
                    loop.tile([128, 128], F32, name="resf", tag="resf", bufs=2),
                    loop.tile([128, 128], F16, name="ZlT", tag="ZlT", bufs=2),
                    loop.tile([128, 128], F16, name="numT", tag="numT", bufs=2),
                )
            resf, ZlT, numT = resf
            for k in ks:
                csl = slice(k * CH, (k + 1) * CH)
                q0 = t * 128 + k * 32

                ps_c1 = psum.tile([128, CH], F32, tag="ps_c1")
                ps_p = psum.tile([128, CH], F32, tag="ps_p")
                ps_1 = psum.tile([128, CH], F32, tag="ps_me")
                rhs_a = (
                    xyzqTb[:, q0 : q0 + 32]
                    .unsqueeze(2)
                    .broadcast_to([3, 32, KNN])
                )
                nc.tensor.matmul(
                    out=ps_c1[:], lhsT=dw1b[:], rhs=rhs_a, start=True, stop=False,
                )
                nc.tensor.matmul(
                    out=ps_c1[:], lhsT=ident_b[:], rhs=G[:, 2, csl],
                    start=False, stop=True,
                )
                nc.tensor.matmul(
                    out=ps_p[:], lhsT=ident_b[:], rhs=G[:, 0, csl],
                    start=True, stop=False,
                )
                nc.tensor.matmul(
                    out=ps_1[:], lhsT=ident_b[:], rhs=G[:, 1, csl],
                    start=True, stop=False,
                )
                # h1 = relu(a + c + db1)
                h1 = loop.tile([128, CH], BF16, tag="h1", bufs=3)
                nc.scalar.activation(out=h1[:], in_=ps_c1[:], func=AF.Relu, bias=db1c[:])
                # y' = dw2.T @ h1 + v ; hat-pre = W2.T @ h1 + w
                nc.tensor.matmul(
                    out=ps_p[:], lhsT=dw2b[:], rhs=h1[:], start=False, stop=True,
                )
                nc.tensor.matmul(
                    out=ps_1[:], lhsT=W2b[:], rhs=h1[:], start=False, stop=True,
                )
                hat = loop.tile([128, CH], BF16, tag="hat", bufs=3)
                nc.scalar.activation(out=hat[:], in_=ps_1[:], func=AF.Relu, bias=gb1uw[:])

                # e = exp(gw2.T @ hat + gb2)
                ps_2 = psum.tile([128, CH], F32, tag="ps_me")
                nc.tensor.matmul(
                    out=ps_2[:], lhsT=gw2b[:], rhs=hat[:], start=True, stop=True,
                )
                e = loop.tile([128, 32, KNN], F16, tag="e", bufs=3)
                nc.scalar.activation(
                    out=e[:].rearrange("p q s -> p (q s)"), in_=ps_2[:],
                    func=AF.Exp, bias=gb2c[:],
                )

                # accumulate per-chunk sums; epilogue batched per tile
                with nc.allow_low_precision(reason="17-term softmax sums in fp16"):
                    nc.vector.tensor_reduce(
                        out=ZlT[:, k * 32 : (k + 1) * 32], in_=e[:], axis=AX.X,
                        op=ALU.add,
                    )
                y16 = loop.tile([128, CH], F16, tag="y16", bufs=3)
                nc.scalar.activation(out=y16[:], in_=ps_p[:], func=AF.Copy)
                nc.vector.tensor_mul(
                    e[:].rearrange("p q s -> p (q s)"),
                    e[:].rearrange("p q s -> p (q s)"),
                    y16[:],
                )
                with nc.allow_low_precision(reason="17-term softmax sums in fp16"):
                    nc.vector.tensor_reduce(
                        out=numT[:, k * 32 : (k + 1) * 32], in_=e[:], axis=AX.X,
                        op=ALU.add,
                    )

            if ks[-1] == NCH - 1:
                # res = (sum_s e*y' + db2*Z + eg*vg) / (Z + eg), whole tile
                num2 = loop.tile([128, 128], F16, tag="num2", bufs=2)
                nc.vector.scalar_tensor_tensor(
                    out=num2[:], in0=ZlT[:], scalar=db2c[:], in1=numT[:],
                    op0=ALU.mult, op1=ALU.add,
                )
                num3 = loop.tile([128, 128], F16, tag="num3", bufs=2)
                nc.vector.tensor_add(num3[:], num2[:], egvg[:].to_broadcast([128, 128]))
                Z2 = loop.tile([128, 128], F16, tag="Z2", bufs=2)
                nc.vector.tensor_add(Z2[:], ZlT[:], eg_c[:].to_broadcast([128, 128]))
                rz = loop.tile([128, 128], F16, tag="rz", bufs=2)
                with nc.allow_low_precision(reason="fp16 softmax denominator"):
                    nc.vector.reciprocal(rz[:], Z2[:])
                nc.vector.tensor_mul(resf[:], num3[:], rz[:])
                # store feature-major output block (host transposes)
                nc.sync.dma_start(out=out[:, t * 128 : (t + 1) * 128], in_=resf[:])
            return (resf, ZlT, numT)

        # software pipeline. Per loop t:
        #   dist/topk(t)  [PE dist + DVE topk]
        #   mlp(t-LA) first half  [PE busy while topk(t) runs on DVE]
        #   idx(t) + gather(t)    [wrapped ready right after topk -> the
        #                          16.5us SWDGE desc-gen + DMA overlap mlp]
        #   mlp(t-LA) second half
        # tile 0's dist/topk is emitted first so DVE overlaps the table build.
        for h in range(2):
            nc.sync.dma_start_transpose(
                out=ptT[h][:], in_=ptbf[:, h * 128 : (h + 1) * 128]
            )
        pend = {0: phase_dist_topk(0), 1: phase_dist_topk(1), 2: phase_dist_topk(2)}
        for nt in range(NT):
            setup_b(nt)
        Gs = {}
        for t in range(NT):
            idxf = pend.pop(t) if t in pend else phase_dist_topk(t)
            rf = None
            if t >= LA:
                rf = phase_mlp(t - LA, Gs[t - LA], (0, 1))
            wrapped = phase_idx(t, idxf)
            Gs[t] = phase_gather(t, wrapped)
            if t >= LA:
                phase_mlp(t - LA, Gs.pop(t - LA), (2, 3), rf)
        for t in range(NT - LA, NT):
            rf = phase_mlp(t, Gs[t], (0, 1))
            phase_mlp(t, Gs.pop(t), (2, 3), rf)

def build_nc():
    nc = bacc.Bacc(
        "TRN2", target_bir_lowering=False, debug=False, enable_asserts=False
    )
    io = {
        "xyz_q": nc.dram_tensor("xyz_q", [QPC, 3], F32, kind="ExternalInput").ap(),
        "xyz": nc.dram_tensor("xyz", [NPT, 3], F32, kind="ExternalInput").ap(),
        "points": nc.dram_tensor("points", [NPT, DI], F32, kind="ExternalInput").ap(),
        "lat": nc.dram_tensor("lat", [1, DI], F32, kind="ExternalInput").ap(),
        "w_qs": nc.dram_tensor("w_qs", [DI, D], F32, kind="ExternalInput").ap(),
        "w_ks": nc.dram_tensor("w_ks", [DI, D], F32, kind="ExternalInput").ap(),
        "w_vs": nc.dram_tensor("w_vs", [DI, D], F32, kind="ExternalInput").ap(),
        "w_kg": nc.dram_tensor("w_kg", [DI, D], F32, kind="ExternalInput").ap(),
        "w_vg": nc.dram_tensor("w_vg", [DI, D], F32, kind="ExternalInput").ap(),
        "dw1": nc.dram_tensor("dw1", [3, D], F32, kind="ExternalInput").ap(),
        "db1": nc.dram_tensor("db1", [1, D], F32, kind="ExternalInput").ap(),
        "dw2": nc.dram_tensor("dw2", [D, D], F32, kind="ExternalInput").ap(),
        "db2": nc.dram_tensor("db2", [1, D], F32, kind="ExternalInput").ap(),
        "gw1": nc.dram_tensor("gw1", [D, D], F32, kind="ExternalInput").ap(),
        "gb1": nc.dram_tensor("gb1", [1, D], F32, kind="ExternalInput").ap(),
        "gw2": nc.dram_tensor("gw2", [D, D], F32, kind="ExternalInput").ap(),
        "gb2": nc.dram_tensor("gb2", [1, D], F32, kind="ExternalInput").ap(),
        "out": nc.dram_tensor("out", [D, QPC], F32, kind="ExternalOutput").ap(),
        "tbl": nc.dram_tensor("tbl", [NPT, TBL_W], BF16).ap(),
        "ptbf": nc.dram_tensor("ptbf", [NPT, DI], BF16).ap(),
    }
    with tile.TileContext(nc) as tc:
        _emit(tc, io)
    nc.compile()
    return nc


def make_in_maps(inputs: dict) -> list[dict]:
    f = lambda x: np.ascontiguousarray(np.asarray(x), dtype=np.float32)
    xyz_q, lat_rep = f(inputs["xyz_q"]), f(inputs["lat_rep"])
    xyz, points = f(inputs["xyz"]), f(inputs["points"])
    w = {k: f(inputs[k]) for k in ("w_qs", "w_ks", "w_vs", "w_kg", "w_vg",
                                   "dw1", "dw2", "gw1", "gw2")}
    bias = {k: f(inputs[k]).reshape(1, D) for k in ("db1", "db2", "gb1", "gb2")}
    in_maps = []
    for core in range(NCORES):
        b, h = core // 2, core % 2
        qsl = slice(h * QPC, (h + 1) * QPC)
        m = {
            "xyz_q": np.ascontiguousarray(xyz_q[b, qsl]),
            "xyz": xyz[b],
            "points": points[b],
            "lat": lat_rep[b : b + 1],
        }
        m.update(w)
        m.update(bias)
        in_maps.append(m)
    return in_maps


_NC = None


def _get_nc():
    global _NC
    if _NC is None:
        _NC = build_nc()
    return _NC


def kernel(**inputs) -> np.ndarray:
    from concourse.bass_utils import run_bass_kernel_spmd

    in_maps = make_in_maps(inputs)
    res = run_bass_kernel_spmd(_get_nc(), in_maps, core_ids=list(range(NCORES)))
    out = np.empty((B, NQ, D), np.float32)
    for core in range(NCORES):
        b, h = core // 2, core % 2
        out[b, h * QPC : (h + 1) * QPC] = np.asarray(res.results[core]["out"]).T
    return out
